# revision 6
# baseline (speedup 1.0000x reference)
"""Kalman CV filter (nn_KalmanCV) — Trainium2 Bass kernel, 8-core data parallel.

Math: the covariance P (and thus the Kalman gains K_t and the output
channels sx/sy/rho) is batch-independent — it depends only on the scalar
inputs. The whole per-batch computation collapses to a linear map over
the 32 history scalars:

    out[l, b, ch<2] = sum_{t,ci} W[t*2+ci, l*2+ch] * hist[t, b, ci]
    out[l, b, ch>=2] = const[l, ch]          (sx, sy, rho — host-filled)

Device kernel per core (all bf16 I/O):
  - 2-block-diagonal weight packing: W2 = blockdiag(Wmu, Wmu) of shape
    (64, 128-padded), so each 512-col matmul tile processes TWO batch
    chunks at once (contraction 64, output partitions 100). Halves the
    columns streamed through the PE.
  - Only the 50 batch-dependent output rows (mu_x/mu_y per step) are
    computed and DMA'd out; the 75 constant rows never touch the device.
  - Output DMA is split into one call per 512-col tile, issued from
    multiple queues, because SBUF->HBM calls pin to a single SDMA engine
    (~25 GB/s each); splitting spreads them across engines.
"""
import numpy as np
import ml_dtypes

DT = 0.2
LEN_HIST = 16
LEN_PRED = 25
BATCH = 100000

N_CORES = 8
TILE = 512                  # matmul free size = one PSUM bank of f32
BLK = BATCH // N_CORES // 2 # 6250 real batch per block (2 blocks/core)
COLS = BLK                  # columns per core (ragged last tile, no padding)
NT = (COLS + TILE - 1) // TILE          # 13 tiles
TILE_COLS = [TILE] * (NT - 1) + [COLS - TILE * (NT - 1)]   # [512]*12 + [106]
TILE_OFF = [TILE * j for j in range(NT)]
# input DMA split: tile counts per call — first tiny so matmul 0 starts early
IN_SPLIT = [1, 4, 4, 4]
K2 = 64                     # packed contraction dim (2 x 32)
M_OUT = 100                 # 2 blocks x 50 mu rows
M_PAD = 128                 # weight free size padded for fast weight load

BF16 = ml_dtypes.bfloat16


def _build_wc(vsx, vsy, asx, asy, GR, coef_G, len_pred):
    """Collapse the filter to W (32, 5L) and constant vector cvec (5L,)."""
    L = int(len_pred)
    H = np.zeros((2, 4)); H[0, 0] = 1.0; H[1, 2] = 1.0
    F = np.eye(4); F[0, 1] = DT; F[2, 3] = DT
    G = np.array([DT * DT / 2, DT, DT * DT / 2, DT])
    Id = np.eye(4)

    ax2 = float(asx[0]) ** 2
    ay2 = float(asy[0]) ** 2
    mx = np.array([1.0, 1.0, 0.0, 0.0]); my = 1.0 - mx
    scale = (ax2 * np.outer(mx, mx) + ay2 * np.outer(my, my)
             + np.outer(mx, my) + np.outer(my, mx))
    g = G * np.tanh(np.asarray(coef_G, np.float64))
    Q = np.outer(g, g) * scale
    R = np.outer(np.asarray(GR, np.float64), np.asarray(GR, np.float64))

    D0 = np.array([[1.0, 0.0], [-1.0 / DT, 0.0], [0.0, 1.0], [0.0, -1.0 / DT]])
    D1 = np.array([[0.0, 0.0], [1.0 / DT, 0.0], [0.0, 0.0], [0.0, 1.0 / DT]])
    P = np.diag([R[0, 0], float(vsx[0]) ** 2, R[1, 1], float(vsy[0]) ** 2])

    C = np.zeros((LEN_HIST, 4, 2))
    C[0] = D0; C[1] = D1
    for t in range(1, LEN_HIST):
        P = F @ P @ F.T + Q
        S = H @ P @ H.T + R
        K = P @ H.T @ np.linalg.inv(S)
        A = (Id - K @ H) @ F
        C = np.einsum('ij,tjk->tik', A, C)
        C[t] += K
        ImKH = Id - K @ H
        P = ImKH @ P @ ImKH.T + K @ R @ K.T

    W_dev = np.zeros((2 * LEN_HIST, 5 * L))
    cvec = np.zeros(5 * L)
    M = np.eye(4)
    for l in range(L):
        M = F @ M
        P = F @ P @ F.T + Q
        HFl = H @ M
        Wl = np.einsum('ij,tjk->itk', HFl, C)   # (2, T, 2)
        for ch in range(2):
            W_dev[:, l * 5 + ch] = Wl[ch].reshape(-1)
        Pout = H @ P @ H.T
        sx = np.sqrt(Pout[0, 0]); sy = np.sqrt(Pout[1, 1])
        cvec[l * 5 + 2] = sx
        cvec[l * 5 + 3] = sy
        cvec[l * 5 + 4] = (Pout[0, 1] + Pout[1, 0]) / (2.0 * sx * sy)
    return W_dev, cvec


_NC_CACHE = {}


def _build_bass():
    import concourse.bass as bass
    import concourse.bacc as bacc
    import concourse.tile as tile
    from concourse import mybir

    nc = bacc.Bacc("TRN2", target_bir_lowering=False, debug=False,
                   num_devices=N_CORES)
    # one dram param per input-DMA call so tile deps stay per-call
    xs = []
    off = 0
    for i, ntile in enumerate(IN_SPLIT):
        ncols = sum(TILE_COLS[off:off + ntile])
        xs.append(nc.declare_dram_parameter(
            f"x{i}", [K2, ncols], mybir.dt.bfloat16, isOutput=False))
        off += ntile
    w = nc.declare_dram_parameter("w", [K2, M_PAD], mybir.dt.bfloat16, isOutput=False)
    out = nc.declare_dram_parameter("out", [M_OUT, COLS], mybir.dt.bfloat16, isOutput=True)

    # out-DMA issuer per tile: gpsimd spreads over 16 SDMA engines,
    # sync/scalar over 10; bias toward gpsimd, keep sync light (it also
    # issues the input DMAs).
    issuers = ["g", "c", "g", "s", "c", "g", "s", "c", "g", "s", "c", "g", "c"]

    with tile.TileContext(nc) as tc:
        with tc.tile_pool(name="singles", bufs=1) as singles, \
             tc.tile_pool(name="ps", bufs=8, space="PSUM") as psum_pool, \
             tc.tile_pool(name="op", bufs=NT) as out_pool:
            x_tiles = []
            # x0 (one tile) first so matmul 0 can start ASAP; then x1, w,
            # then the rest — all on sync.
            t0 = singles.tile([K2, xs[0].shape[1]], mybir.dt.bfloat16)
            nc.sync.dma_start(out=t0, in_=xs[0][:, :])
            x_tiles.append(t0)
            t1 = singles.tile([K2, xs[1].shape[1]], mybir.dt.bfloat16)
            nc.sync.dma_start(out=t1, in_=xs[1][:, :])
            x_tiles.append(t1)
            w_tile = singles.tile([K2, M_PAD], mybir.dt.bfloat16)
            nc.sync.dma_start(out=w_tile, in_=w[:, :])
            for i in range(2, len(IN_SPLIT)):
                ti = singles.tile([K2, xs[i].shape[1]], mybir.dt.bfloat16)
                nc.sync.dma_start(out=ti, in_=xs[i][:, :])
                x_tiles.append(ti)

            # map tile j -> (input call index, col offset within it)
            tile_src = []
            off = 0
            for i, ntile in enumerate(IN_SPLIT):
                for k in range(ntile):
                    tile_src.append((i, sum(TILE_COLS[off:off + k])))
                off += ntile

            n_scalar_copy = 0
            for j in range(NT):
                tc_j = TILE_COLS[j]
                src_i, src_off = tile_src[j]
                x_sl = x_tiles[src_i][:, src_off:src_off + tc_j]
                ps = psum_pool.tile([M_PAD, tc_j], mybir.dt.float32)
                nc.tensor.matmul(ps, w_tile, x_sl, start=True, stop=True)
                o_tile = out_pool.tile([M_OUT, tc_j], mybir.dt.bfloat16)
                if j % 3 == 1:
                    nc.scalar.copy(out=o_tile, in_=ps[:M_OUT, :])
                    n_scalar_copy += 1
                else:
                    nc.vector.tensor_scalar_add(o_tile, ps[:M_OUT, :], 0.0)
                issuer = {"s": nc.sync, "c": nc.scalar, "g": nc.gpsimd}[issuers[j]]
                issuer.dma_start(
                    out=out[:, TILE_OFF[j]:TILE_OFF[j] + tc_j], in_=o_tile)
    nc.compile()
    return nc


def _get_nc():
    if "nc" not in _NC_CACHE:
        _NC_CACHE["nc"] = _build_bass()
    return _NC_CACHE["nc"]


def _pack_inputs(hist_T_bf, W2):
    """Per-core input dicts: block-packed (64, COLS) bf16, split per DMA call."""
    per_core = BATCH // N_CORES
    in_maps = []
    splits = []
    off = 0
    for ntile in IN_SPLIT:
        ncols = sum(TILE_COLS[off:off + ntile])
        splits.append((TILE_OFF[off], ncols))
        off += ntile
    for c in range(N_CORES):
        x2 = np.empty((K2, COLS), dtype=BF16)
        base = c * per_core
        x2[:32] = hist_T_bf[:, base:base + BLK]
        x2[32:] = hist_T_bf[:, base + BLK:base + 2 * BLK]
        m = {"w": W2}
        for i, (o, n) in enumerate(splits):
            m[f"x{i}"] = np.ascontiguousarray(x2[:, o:o + n])
        in_maps.append(m)
    return in_maps


def _run_device(hist_T_bf, W2, trace=False):
    from concourse.bass_utils import run_bass_kernel_spmd
    in_maps = _pack_inputs(hist_T_bf, W2)
    return run_bass_kernel_spmd(_get_nc(), in_maps, list(range(N_CORES)),
                                trace=trace)


def _pack_w(W_dev, L):
    """(32, 5L) f64 -> blockdiag-packed (64, M_PAD) bf16, mu rows only."""
    Wmu = np.zeros((32, 2 * L))
    for l in range(L):
        Wmu[:, 2 * l] = W_dev[:, 5 * l]
        Wmu[:, 2 * l + 1] = W_dev[:, 5 * l + 1]
    W2 = np.zeros((K2, M_PAD), dtype=BF16)
    W2[:32, :2 * L] = Wmu.astype(BF16)
    W2[32:, 2 * L:4 * L] = Wmu.astype(BF16)
    return W2


def kernel(hist, velocity_std_x, velocity_std_y, acceleration_std_x,
           acceleration_std_y, GR, coef_G, len_pred):
    hist = np.asarray(hist, np.float32)
    L = int(len_pred)
    W_dev, cvec = _build_wc(velocity_std_x, velocity_std_y, acceleration_std_x,
                            acceleration_std_y, GR, coef_G, L)
    T, B, _ = hist.shape
    hist_T = np.ascontiguousarray(hist.transpose(0, 2, 1)).reshape(2 * T, B)

    if L != LEN_PRED or B != BATCH or T != LEN_HIST:
        # shape surprise: fall back to exact host math
        out_flat = W_dev.astype(np.float32).T @ hist_T \
            + cvec.astype(np.float32)[:, None]
        return np.ascontiguousarray(
            out_flat.reshape(L, 5, B).transpose(0, 2, 1)).astype(np.float32)

    W2 = _pack_w(W_dev, L)
    hist_T_bf = hist_T.astype(BF16)
    res = _run_device(hist_T_bf, W2)

    per_core = B // N_CORES
    out = np.empty((L, B, 5), np.float32)
    # constant channels: sx, sy, rho
    out[:, :, 2] = cvec[2::5].astype(np.float32)[:, None]
    out[:, :, 3] = cvec[3::5].astype(np.float32)[:, None]
    out[:, :, 4] = cvec[4::5].astype(np.float32)[:, None]
    for c in range(N_CORES):
        oc = np.asarray(res.results[c]["out"]).astype(np.float32)  # (100, COLS)
        base = c * per_core
        # block A: rows 0:50 -> (25, 2, BLK) -> (25, BLK, 2)
        out[:, base:base + BLK, :2] = (
            oc[:50].reshape(L, 2, BLK).transpose(0, 2, 1))
        out[:, base + BLK:base + 2 * BLK, :2] = (
            oc[50:100].reshape(L, 2, BLK).transpose(0, 2, 1))
    return out


# revision 7
# speedup vs baseline: 1.0589x; 1.0589x over previous
"""Kalman CV filter (nn_KalmanCV) — Trainium2 Bass kernel, 8-core data parallel.

Math: the covariance P (and thus the Kalman gains K_t and the output
channels sx/sy/rho) is batch-independent — it depends only on the scalar
inputs. The whole per-batch computation collapses to a linear map over
the 32 history scalars:

    out[l, b, ch<2] = sum_{t,ci} W[t*2+ci, l*2+ch] * hist[t, b, ci]
    out[l, b, ch>=2] = const[l, ch]          (sx, sy, rho — host-filled)

Device kernel per core (all bf16 I/O):
  - 2-block-diagonal weight packing: W2 = blockdiag(Wmu, Wmu) of shape
    (64, 128-padded), so each 512-col matmul tile processes TWO batch
    chunks at once (contraction 64, output partitions 100). Halves the
    columns streamed through the PE.
  - Only the 50 batch-dependent output rows (mu_x/mu_y per step) are
    computed and DMA'd out; the 75 constant rows never touch the device.
  - Output DMA is split into one call per 512-col tile, issued from
    multiple queues, because SBUF->HBM calls pin to a single SDMA engine
    (~25 GB/s each); splitting spreads them across engines.
"""
import numpy as np
import ml_dtypes

DT = 0.2
LEN_HIST = 16
LEN_PRED = 25
BATCH = 100000

N_CORES = 8
TILE = 512                  # matmul free size = one PSUM bank of f32
BLK = BATCH // N_CORES // 2 # 6250 real batch per block (2 blocks/core)
COLS = BLK                  # columns per core (ragged last tile, no padding)
NT = (COLS + TILE - 1) // TILE          # 13 tiles
TILE_COLS = [TILE] * (NT - 1) + [COLS - TILE * (NT - 1)]   # [512]*12 + [106]
TILE_OFF = [TILE * j for j in range(NT)]
# input DMA split: tile counts per call — first tiny so matmul 0 starts early
IN_SPLIT = [1, 4, 4, 4]
K2 = 64                     # packed contraction dim (2 x 32)
M_OUT = 100                 # 2 blocks x 50 mu rows
M_PAD = 128                 # weight free size padded for fast weight load

BF16 = ml_dtypes.bfloat16


def _build_wc(vsx, vsy, asx, asy, GR, coef_G, len_pred):
    """Collapse the filter to W (32, 5L) and constant vector cvec (5L,)."""
    L = int(len_pred)
    H = np.zeros((2, 4)); H[0, 0] = 1.0; H[1, 2] = 1.0
    F = np.eye(4); F[0, 1] = DT; F[2, 3] = DT
    G = np.array([DT * DT / 2, DT, DT * DT / 2, DT])
    Id = np.eye(4)

    ax2 = float(asx[0]) ** 2
    ay2 = float(asy[0]) ** 2
    mx = np.array([1.0, 1.0, 0.0, 0.0]); my = 1.0 - mx
    scale = (ax2 * np.outer(mx, mx) + ay2 * np.outer(my, my)
             + np.outer(mx, my) + np.outer(my, mx))
    g = G * np.tanh(np.asarray(coef_G, np.float64))
    Q = np.outer(g, g) * scale
    R = np.outer(np.asarray(GR, np.float64), np.asarray(GR, np.float64))

    D0 = np.array([[1.0, 0.0], [-1.0 / DT, 0.0], [0.0, 1.0], [0.0, -1.0 / DT]])
    D1 = np.array([[0.0, 0.0], [1.0 / DT, 0.0], [0.0, 0.0], [0.0, 1.0 / DT]])
    P = np.diag([R[0, 0], float(vsx[0]) ** 2, R[1, 1], float(vsy[0]) ** 2])

    C = np.zeros((LEN_HIST, 4, 2))
    C[0] = D0; C[1] = D1
    for t in range(1, LEN_HIST):
        P = F @ P @ F.T + Q
        S = H @ P @ H.T + R
        K = P @ H.T @ np.linalg.inv(S)
        A = (Id - K @ H) @ F
        C = np.einsum('ij,tjk->tik', A, C)
        C[t] += K
        ImKH = Id - K @ H
        P = ImKH @ P @ ImKH.T + K @ R @ K.T

    W_dev = np.zeros((2 * LEN_HIST, 5 * L))
    cvec = np.zeros(5 * L)
    M = np.eye(4)
    for l in range(L):
        M = F @ M
        P = F @ P @ F.T + Q
        HFl = H @ M
        Wl = np.einsum('ij,tjk->itk', HFl, C)   # (2, T, 2)
        for ch in range(2):
            W_dev[:, l * 5 + ch] = Wl[ch].reshape(-1)
        Pout = H @ P @ H.T
        sx = np.sqrt(Pout[0, 0]); sy = np.sqrt(Pout[1, 1])
        cvec[l * 5 + 2] = sx
        cvec[l * 5 + 3] = sy
        cvec[l * 5 + 4] = (Pout[0, 1] + Pout[1, 0]) / (2.0 * sx * sy)
    return W_dev, cvec


_NC_CACHE = {}


def _build_bass():
    import concourse.bass as bass
    import concourse.bacc as bacc
    import concourse.tile as tile
    from concourse import mybir

    nc = bacc.Bacc("TRN2", target_bir_lowering=False, debug=False,
                   num_devices=N_CORES)
    # one dram param per input-DMA call so tile deps stay per-call
    xs = []
    off = 0
    for i, ntile in enumerate(IN_SPLIT):
        ncols = sum(TILE_COLS[off:off + ntile])
        xs.append(nc.declare_dram_parameter(
            f"x{i}", [K2, ncols], mybir.dt.bfloat16, isOutput=False))
        off += ntile
    w = nc.declare_dram_parameter("w", [K2, M_PAD], mybir.dt.bfloat16, isOutput=False)
    out = nc.declare_dram_parameter("out", [M_OUT, COLS], mybir.dt.bfloat16, isOutput=True)

    # out-DMA issuer per tile: gpsimd spreads over 16 SDMA engines,
    # sync/scalar over 10; bias toward gpsimd, keep sync light (it also
    # issues the input DMAs).
    issuers = ["g", "c", "g", "s", "c", "g", "s", "c", "g", "s", "c", "g", "c"]

    with tile.TileContext(nc) as tc:
        with tc.tile_pool(name="singles", bufs=1) as singles, \
             tc.tile_pool(name="ps", bufs=8, space="PSUM") as psum_pool, \
             tc.tile_pool(name="op", bufs=NT) as out_pool:
            # w first so LDWEIGHTS is ready by the time x0 lands; distinct
            # tags so singles tiles get their own slots (default tag shares
            # one slot ring and serializes the whole pipeline).
            w_tile = singles.tile([K2, M_PAD], mybir.dt.bfloat16, tag="w")
            nc.sync.dma_start(out=w_tile, in_=w[:, :])
            x_tiles = []
            for i in range(len(IN_SPLIT)):
                ti = singles.tile([K2, xs[i].shape[1]], mybir.dt.bfloat16,
                                  tag=f"x{i}")
                nc.sync.dma_start(out=ti, in_=xs[i][:, :])
                x_tiles.append(ti)

            # map tile j -> (input call index, col offset within it)
            tile_src = []
            off = 0
            for i, ntile in enumerate(IN_SPLIT):
                for k in range(ntile):
                    tile_src.append((i, sum(TILE_COLS[off:off + k])))
                off += ntile

            n_scalar_copy = 0
            for j in range(NT):
                tc_j = TILE_COLS[j]
                src_i, src_off = tile_src[j]
                x_sl = x_tiles[src_i][:, src_off:src_off + tc_j]
                ps = psum_pool.tile([M_PAD, tc_j], mybir.dt.float32)
                nc.tensor.matmul(ps, w_tile, x_sl, start=True, stop=True)
                o_tile = out_pool.tile([M_OUT, tc_j], mybir.dt.bfloat16)
                if j % 3 == 1:
                    nc.scalar.copy(out=o_tile, in_=ps[:M_OUT, :])
                    n_scalar_copy += 1
                else:
                    nc.vector.tensor_scalar_add(o_tile, ps[:M_OUT, :], 0.0)
                issuer = {"s": nc.sync, "c": nc.scalar, "g": nc.gpsimd}[issuers[j]]
                issuer.dma_start(
                    out=out[:, TILE_OFF[j]:TILE_OFF[j] + tc_j], in_=o_tile)
    nc.compile()
    return nc


def _get_nc():
    if "nc" not in _NC_CACHE:
        _NC_CACHE["nc"] = _build_bass()
    return _NC_CACHE["nc"]


def _pack_inputs(hist_T_bf, W2):
    """Per-core input dicts: block-packed (64, COLS) bf16, split per DMA call."""
    per_core = BATCH // N_CORES
    in_maps = []
    splits = []
    off = 0
    for ntile in IN_SPLIT:
        ncols = sum(TILE_COLS[off:off + ntile])
        splits.append((TILE_OFF[off], ncols))
        off += ntile
    for c in range(N_CORES):
        x2 = np.empty((K2, COLS), dtype=BF16)
        base = c * per_core
        x2[:32] = hist_T_bf[:, base:base + BLK]
        x2[32:] = hist_T_bf[:, base + BLK:base + 2 * BLK]
        m = {"w": W2}
        for i, (o, n) in enumerate(splits):
            m[f"x{i}"] = np.ascontiguousarray(x2[:, o:o + n])
        in_maps.append(m)
    return in_maps


def _run_device(hist_T_bf, W2, trace=False):
    from concourse.bass_utils import run_bass_kernel_spmd
    in_maps = _pack_inputs(hist_T_bf, W2)
    return run_bass_kernel_spmd(_get_nc(), in_maps, list(range(N_CORES)),
                                trace=trace)


def _pack_w(W_dev, L):
    """(32, 5L) f64 -> blockdiag-packed (64, M_PAD) bf16, mu rows only."""
    Wmu = np.zeros((32, 2 * L))
    for l in range(L):
        Wmu[:, 2 * l] = W_dev[:, 5 * l]
        Wmu[:, 2 * l + 1] = W_dev[:, 5 * l + 1]
    W2 = np.zeros((K2, M_PAD), dtype=BF16)
    W2[:32, :2 * L] = Wmu.astype(BF16)
    W2[32:, 2 * L:4 * L] = Wmu.astype(BF16)
    return W2


def kernel(hist, velocity_std_x, velocity_std_y, acceleration_std_x,
           acceleration_std_y, GR, coef_G, len_pred):
    hist = np.asarray(hist, np.float32)
    L = int(len_pred)
    W_dev, cvec = _build_wc(velocity_std_x, velocity_std_y, acceleration_std_x,
                            acceleration_std_y, GR, coef_G, L)
    T, B, _ = hist.shape
    hist_T = np.ascontiguousarray(hist.transpose(0, 2, 1)).reshape(2 * T, B)

    if L != LEN_PRED or B != BATCH or T != LEN_HIST:
        # shape surprise: fall back to exact host math
        out_flat = W_dev.astype(np.float32).T @ hist_T \
            + cvec.astype(np.float32)[:, None]
        return np.ascontiguousarray(
            out_flat.reshape(L, 5, B).transpose(0, 2, 1)).astype(np.float32)

    W2 = _pack_w(W_dev, L)
    hist_T_bf = hist_T.astype(BF16)
    res = _run_device(hist_T_bf, W2)

    per_core = B // N_CORES
    out = np.empty((L, B, 5), np.float32)
    # constant channels: sx, sy, rho
    out[:, :, 2] = cvec[2::5].astype(np.float32)[:, None]
    out[:, :, 3] = cvec[3::5].astype(np.float32)[:, None]
    out[:, :, 4] = cvec[4::5].astype(np.float32)[:, None]
    for c in range(N_CORES):
        oc = np.asarray(res.results[c]["out"]).astype(np.float32)  # (100, COLS)
        base = c * per_core
        # block A: rows 0:50 -> (25, 2, BLK) -> (25, BLK, 2)
        out[:, base:base + BLK, :2] = (
            oc[:50].reshape(L, 2, BLK).transpose(0, 2, 1))
        out[:, base + BLK:base + 2 * BLK, :2] = (
            oc[50:100].reshape(L, 2, BLK).transpose(0, 2, 1))
    return out


# revision 8
# speedup vs baseline: 1.1151x; 1.0531x over previous
"""Kalman CV filter (nn_KalmanCV) — Trainium2 Bass kernel, 8-core data parallel.

Math: the covariance P (and thus the Kalman gains K_t and the output
channels sx/sy/rho) is batch-independent — it depends only on the scalar
inputs. The whole per-batch computation collapses to a linear map over
the 32 history scalars:

    out[l, b, ch<2] = sum_{t,ci} W[t*2+ci, l*2+ch] * hist[t, b, ci]
    out[l, b, ch>=2] = const[l, ch]          (sx, sy, rho — host-filled)

Device kernel per core (all bf16 I/O):
  - 2-block-diagonal weight packing: W2 = blockdiag(Wmu, Wmu) of shape
    (64, 128-padded), so each 512-col matmul tile processes TWO batch
    chunks at once (contraction 64, output partitions 100). Halves the
    columns streamed through the PE.
  - Only the 50 batch-dependent output rows (mu_x/mu_y per step) are
    computed and DMA'd out; the 75 constant rows never touch the device.
  - Output DMA is split into one call per 512-col tile, issued from
    multiple queues, because SBUF->HBM calls pin to a single SDMA engine
    (~25 GB/s each); splitting spreads them across engines.
"""
import numpy as np
import ml_dtypes

DT = 0.2
LEN_HIST = 16
LEN_PRED = 25
BATCH = 100000

N_CORES = 8
TILE = 512                  # matmul free size = one PSUM bank of f32
BLK = BATCH // N_CORES // 2 # 6250 real batch per block (2 blocks/core)
COLS = BLK                  # columns per core (ragged last tile, no padding)
NT = (COLS + TILE - 1) // TILE          # 13 tiles
TILE_COLS = [TILE] * (NT - 1) + [COLS - TILE * (NT - 1)]   # [512]*12 + [106]
TILE_OFF = [TILE * j for j in range(NT)]
# input DMA split: tile counts per call — first tiny so matmul 0 starts early
IN_SPLIT = [1, 4, 4, 4]
K2 = 64                     # packed contraction dim (2 x 32)
M_OUT = 100                 # 2 blocks x 50 mu rows
M_PAD = 128                 # weight free size padded for fast weight load

BF16 = ml_dtypes.bfloat16


def _build_wc(vsx, vsy, asx, asy, GR, coef_G, len_pred):
    """Collapse the filter to W (32, 5L) and constant vector cvec (5L,)."""
    L = int(len_pred)
    H = np.zeros((2, 4)); H[0, 0] = 1.0; H[1, 2] = 1.0
    F = np.eye(4); F[0, 1] = DT; F[2, 3] = DT
    G = np.array([DT * DT / 2, DT, DT * DT / 2, DT])
    Id = np.eye(4)

    ax2 = float(asx[0]) ** 2
    ay2 = float(asy[0]) ** 2
    mx = np.array([1.0, 1.0, 0.0, 0.0]); my = 1.0 - mx
    scale = (ax2 * np.outer(mx, mx) + ay2 * np.outer(my, my)
             + np.outer(mx, my) + np.outer(my, mx))
    g = G * np.tanh(np.asarray(coef_G, np.float64))
    Q = np.outer(g, g) * scale
    R = np.outer(np.asarray(GR, np.float64), np.asarray(GR, np.float64))

    D0 = np.array([[1.0, 0.0], [-1.0 / DT, 0.0], [0.0, 1.0], [0.0, -1.0 / DT]])
    D1 = np.array([[0.0, 0.0], [1.0 / DT, 0.0], [0.0, 0.0], [0.0, 1.0 / DT]])
    P = np.diag([R[0, 0], float(vsx[0]) ** 2, R[1, 1], float(vsy[0]) ** 2])

    C = np.zeros((LEN_HIST, 4, 2))
    C[0] = D0; C[1] = D1
    for t in range(1, LEN_HIST):
        P = F @ P @ F.T + Q
        S = H @ P @ H.T + R
        K = P @ H.T @ np.linalg.inv(S)
        A = (Id - K @ H) @ F
        C = np.einsum('ij,tjk->tik', A, C)
        C[t] += K
        ImKH = Id - K @ H
        P = ImKH @ P @ ImKH.T + K @ R @ K.T

    W_dev = np.zeros((2 * LEN_HIST, 5 * L))
    cvec = np.zeros(5 * L)
    M = np.eye(4)
    for l in range(L):
        M = F @ M
        P = F @ P @ F.T + Q
        HFl = H @ M
        Wl = np.einsum('ij,tjk->itk', HFl, C)   # (2, T, 2)
        for ch in range(2):
            W_dev[:, l * 5 + ch] = Wl[ch].reshape(-1)
        Pout = H @ P @ H.T
        sx = np.sqrt(Pout[0, 0]); sy = np.sqrt(Pout[1, 1])
        cvec[l * 5 + 2] = sx
        cvec[l * 5 + 3] = sy
        cvec[l * 5 + 4] = (Pout[0, 1] + Pout[1, 0]) / (2.0 * sx * sy)
    return W_dev, cvec


_NC_CACHE = {}


def _build_bass():
    import concourse.bass as bass
    import concourse.bacc as bacc
    import concourse.tile as tile
    from concourse import mybir

    nc = bacc.Bacc("TRN2", target_bir_lowering=False, debug=False,
                   num_devices=N_CORES)
    # one dram param per input-DMA call so tile deps stay per-call
    xs = []
    off = 0
    for i, ntile in enumerate(IN_SPLIT):
        ncols = sum(TILE_COLS[off:off + ntile])
        xs.append(nc.declare_dram_parameter(
            f"x{i}", [K2, ncols], mybir.dt.bfloat16, isOutput=False))
        off += ntile
    w = nc.declare_dram_parameter("w", [K2, M_PAD], mybir.dt.bfloat16, isOutput=False)
    out = nc.declare_dram_parameter("out", [M_OUT, COLS], mybir.dt.bfloat16, isOutput=True)

    # out-DMA issuer per tile: gpsimd spreads over 16 SDMA engines,
    # sync/scalar over 10; bias toward gpsimd, keep sync light (it also
    # issues the input DMAs).
    issuers = ["c", "s", "c", "s", "c", "s", "c", "s", "c", "s", "c", "s", "c"]

    with tile.TileContext(nc) as tc:
        with tc.tile_pool(name="singles", bufs=1) as singles, \
             tc.tile_pool(name="ps", bufs=8, space="PSUM") as psum_pool, \
             tc.tile_pool(name="op", bufs=NT) as out_pool:
            # w first so LDWEIGHTS is ready by the time x0 lands; distinct
            # tags so singles tiles get their own slots (default tag shares
            # one slot ring and serializes the whole pipeline).
            w_tile = singles.tile([K2, M_PAD], mybir.dt.bfloat16, tag="w")
            nc.sync.dma_start(out=w_tile, in_=w[:, :])
            x_tiles = []
            for i in range(len(IN_SPLIT)):
                ti = singles.tile([K2, xs[i].shape[1]], mybir.dt.bfloat16,
                                  tag=f"x{i}")
                nc.sync.dma_start(out=ti, in_=xs[i][:, :])
                x_tiles.append(ti)

            # map tile j -> (input call index, col offset within it)
            tile_src = []
            off = 0
            for i, ntile in enumerate(IN_SPLIT):
                for k in range(ntile):
                    tile_src.append((i, sum(TILE_COLS[off:off + k])))
                off += ntile

            n_scalar_copy = 0
            for j in range(NT):
                tc_j = TILE_COLS[j]
                src_i, src_off = tile_src[j]
                x_sl = x_tiles[src_i][:, src_off:src_off + tc_j]
                ps = psum_pool.tile([M_PAD, tc_j], mybir.dt.float32)
                nc.tensor.matmul(ps, w_tile, x_sl, start=True, stop=True)
                o_tile = out_pool.tile([M_OUT, tc_j], mybir.dt.bfloat16)
                if j % 3 == 1:
                    nc.scalar.copy(out=o_tile, in_=ps[:M_OUT, :])
                    n_scalar_copy += 1
                else:
                    nc.vector.tensor_scalar_add(o_tile, ps[:M_OUT, :], 0.0)
                issuer = {"s": nc.sync, "c": nc.scalar, "g": nc.gpsimd}[issuers[j]]
                issuer.dma_start(
                    out=out[:, TILE_OFF[j]:TILE_OFF[j] + tc_j], in_=o_tile)
    nc.compile()
    return nc


def _get_nc():
    if "nc" not in _NC_CACHE:
        _NC_CACHE["nc"] = _build_bass()
    return _NC_CACHE["nc"]


def _pack_inputs(hist_T_bf, W2):
    """Per-core input dicts: block-packed (64, COLS) bf16, split per DMA call."""
    per_core = BATCH // N_CORES
    in_maps = []
    splits = []
    off = 0
    for ntile in IN_SPLIT:
        ncols = sum(TILE_COLS[off:off + ntile])
        splits.append((TILE_OFF[off], ncols))
        off += ntile
    for c in range(N_CORES):
        x2 = np.empty((K2, COLS), dtype=BF16)
        base = c * per_core
        x2[:32] = hist_T_bf[:, base:base + BLK]
        x2[32:] = hist_T_bf[:, base + BLK:base + 2 * BLK]
        m = {"w": W2}
        for i, (o, n) in enumerate(splits):
            m[f"x{i}"] = np.ascontiguousarray(x2[:, o:o + n])
        in_maps.append(m)
    return in_maps


def _run_device(hist_T_bf, W2, trace=False):
    from concourse.bass_utils import run_bass_kernel_spmd
    in_maps = _pack_inputs(hist_T_bf, W2)
    return run_bass_kernel_spmd(_get_nc(), in_maps, list(range(N_CORES)),
                                trace=trace)


def _pack_w(W_dev, L):
    """(32, 5L) f64 -> blockdiag-packed (64, M_PAD) bf16, mu rows only."""
    Wmu = np.zeros((32, 2 * L))
    for l in range(L):
        Wmu[:, 2 * l] = W_dev[:, 5 * l]
        Wmu[:, 2 * l + 1] = W_dev[:, 5 * l + 1]
    W2 = np.zeros((K2, M_PAD), dtype=BF16)
    W2[:32, :2 * L] = Wmu.astype(BF16)
    W2[32:, 2 * L:4 * L] = Wmu.astype(BF16)
    return W2


def kernel(hist, velocity_std_x, velocity_std_y, acceleration_std_x,
           acceleration_std_y, GR, coef_G, len_pred):
    hist = np.asarray(hist, np.float32)
    L = int(len_pred)
    W_dev, cvec = _build_wc(velocity_std_x, velocity_std_y, acceleration_std_x,
                            acceleration_std_y, GR, coef_G, L)
    T, B, _ = hist.shape
    hist_T = np.ascontiguousarray(hist.transpose(0, 2, 1)).reshape(2 * T, B)

    if L != LEN_PRED or B != BATCH or T != LEN_HIST:
        # shape surprise: fall back to exact host math
        out_flat = W_dev.astype(np.float32).T @ hist_T \
            + cvec.astype(np.float32)[:, None]
        return np.ascontiguousarray(
            out_flat.reshape(L, 5, B).transpose(0, 2, 1)).astype(np.float32)

    W2 = _pack_w(W_dev, L)
    hist_T_bf = hist_T.astype(BF16)
    res = _run_device(hist_T_bf, W2)

    per_core = B // N_CORES
    out = np.empty((L, B, 5), np.float32)
    # constant channels: sx, sy, rho
    out[:, :, 2] = cvec[2::5].astype(np.float32)[:, None]
    out[:, :, 3] = cvec[3::5].astype(np.float32)[:, None]
    out[:, :, 4] = cvec[4::5].astype(np.float32)[:, None]
    for c in range(N_CORES):
        oc = np.asarray(res.results[c]["out"]).astype(np.float32)  # (100, COLS)
        base = c * per_core
        # block A: rows 0:50 -> (25, 2, BLK) -> (25, BLK, 2)
        out[:, base:base + BLK, :2] = (
            oc[:50].reshape(L, 2, BLK).transpose(0, 2, 1))
        out[:, base + BLK:base + 2 * BLK, :2] = (
            oc[50:100].reshape(L, 2, BLK).transpose(0, 2, 1))
    return out
